# revision 28
# baseline (speedup 1.0000x reference)
"""Contrastive-loss kernel for Trainium2 (8 NeuronCores, SPMD data-parallel).

Math (from the reference):
    diag_A_is = (A_is_t + A_is_t_14 + A_is_t_28)[i, i, :]        # [B, D]
    diag_A_em = (A_em_t + A_em_t_14 + A_em_t_28)[i, i, :]        # [B, D]
    loss = sum_b relu( sum_d (0.4*m + 0.6*tr_m) * (diag_A_is - diag_A_em) )

Only the diagonals A[i, i, :] of the six [B, B, D] tensors are touched
(1/256th of the data).  Sharding strategy: batch-dim data parallel across
the 8 cores — the host gathers the diagonal rows (pure data movement) and
ships each core its 32 rows of the eight [B, D] operands; all arithmetic
runs on-device.  Per-core partial losses are summed on the host (8
scalars).

Precision plan (tolerance 2e-2): everything ships as fp8 e4m3 except the
exact-in-bf16 E constants (whole-pipeline rel err 4.2e-3, host-measured
and HW-verified; gate margin ~4.8x).  The DVE reads fp8 directly — mixed
fp8 x bf16 operands run at full DVE rate (measured).  All accumulation is
fp32 (DVE accumulator / PSUM).

Device-side layout per core (all HWDGE; the two rings pipeline; the sync
ring q1 starts ~0.35 us earlier after body entry and carries the
latency-critical W load, the scalar ring q10 drains faster and carries
the bulk):
  sync ring q1:     wmt [128, 512] = m | tr_m      (fp8, 512 B rows)
                    et  [128, 64]  = E | En        (bf16, 128 B rows)
                    ... out store (4 B) at the end
  scalar ring q10:  ais [128, 768] = is0|is1|is2   (fp8, 768 B rows)
                    aem [128, 768] = em0|em1|em2   (fp8, 768 B rows)
  each [32, 1024] operand block is flattened row-major to [128, 256]
  (partition p = 4*row + quarter, 256 contiguous d's per partition).
  E[p, b] = 1.0 iff p // 4 == b; En = -E.  Both are host CONSTANTS (not
  input data): matmul rhs that folds the four per-partition quarter-row
  dots of each batch row (partition reduction), with En carrying the
  is-minus-em sign so no negated weight vector is ever needed.

Factoring: 0.4*m + 0.6*tr_m = 0.4 * (m + 1.5*tr_m) and
relu(0.4 x) = 0.4 relu(x), so the 0.4 is applied host-side to the scalar.
Linearity: the is- and em-blocks are separately dotted against w
(broadcast across the three tensors via a stride-0 AP — same DVE rate as
linear, measured) in one accumulator pass each:
    rowq[:, 0] = sum_f ais * [w,w,w],   rowq[:, 1] = sum_f aem * [w,w,w]
    ps[1, 32]  = rowq[:, 0]^T @ E  +  rowq[:, 1]^T @ En
This removes the -w preparation from the serial DVE chain entirely: the
chain is w-prep, dot(is), dot(em), relu — everything else overlaps.

relu+sum runs on DVE (the Scalar engine's ACT path lazily loads a 1.3 us
function table on first use — measured, avoid).  The 4-byte result store
is issued from the sync ring (cheaper enqueue there; enqueue+flight is
~1.6 us on either ring).

Raw bass (no TileContext) on purpose: this walrus build enforces a tiny
per-instruction sync-wait limit and Tile's epilogue barrier costs several
microseconds.  Engines pipeline, so a same-engine consumer of an earlier
op's output still needs a semaphore edge (the race detector enforces it).
The Pool engine cannot run TensorScalarPtr (codegen engine check) and
SWDGE completion semaphores lag ~1 us behind the last byte, so everything
stays on the two HWDGE rings.
"""

import numpy as np
import ml_dtypes

import concourse.bass as bass
import concourse.mybir as mybir
from concourse.bass_utils import run_bass_kernel_spmd

B = 256
D = 1024
N_CORES = 8
ROWS_PER_CORE = B // N_CORES  # 32
BLK = 256  # free-dim width of one packed [32, 1024] operand block
E_COLS = ROWS_PER_CORE  # 32
FREE_A = 3 * BLK  # 768: is or em block

_NC_CACHE = None


def build_nc() -> bass.Bass:
    f32 = mybir.dt.float32
    bf16 = mybir.dt.bfloat16
    f8 = mybir.dt.float8e4
    Alu = mybir.AluOpType

    nc = bass.Bass()
    xw = nc.dram_tensor("xw", [128 * 2 * BLK], f8, kind="ExternalInput")
    xe = nc.dram_tensor("xe", [128 * 2 * E_COLS], bf16, kind="ExternalInput")
    xis = nc.dram_tensor("xis", [128 * FREE_A], f8, kind="ExternalInput")
    xem = nc.dram_tensor("xem", [128 * FREE_A], f8, kind="ExternalInput")
    out_d = nc.dram_tensor("out", [1, 1], f32, kind="ExternalOutput")

    with (
        nc.sbuf_tensor("wmt", [128, 2 * BLK], f8) as wmt,
        nc.sbuf_tensor("et", [128, 2 * E_COLS], bf16) as et,
        nc.sbuf_tensor("ais", [128, FREE_A], f8) as ais,
        nc.sbuf_tensor("aem", [128, FREE_A], f8) as aem,
        nc.sbuf_tensor("w2", [128, BLK], bf16) as w2,
        nc.sbuf_tensor("prod", [128, 2 * FREE_A], bf16) as prod,
        nc.sbuf_tensor("rowq", [128, 2], bf16) as rowq,
        nc.sbuf_tensor("srelu", [1, E_COLS], f32) as srelu,
        nc.sbuf_tensor("total", [1, 1], f32) as total,
        nc.psum_tensor("ps", [1, E_COLS], f32) as ps,
        nc.semaphore("st") as st,  # scalar ring: m|tr (16), out (32)
        nc.semaphore("sE") as sE,  # scalar ring: E|En load
        nc.semaphore("se") as se,  # sync ring: em block
        nc.semaphore("si") as si,  # sync ring: is block
        nc.semaphore("vs") as vs,  # vector progress
        nc.semaphore("pe") as pe,  # tensor: partition fold done
        nc.Block() as block,
    ):
        m_ap = wmt[:, 0:BLK]
        tr_ap = wmt[:, BLK : 2 * BLK]
        e_ap = et[:, 0:E_COLS]
        en_ap = et[:, E_COLS : 2 * E_COLS]
        w_b = w2[:, :].unsqueeze(1).broadcast_to([128, 3, BLK])

        @block.sync
        def _(sync):
            sync.dma_start(
                out=wmt[:, :], in_=xw[:].rearrange("(p f) -> p f", f=2 * BLK)
            ).then_inc(st, 16)
            sync.dma_start(
                out=et[:, :], in_=xe[:].rearrange("(p f) -> p f", f=2 * E_COLS)
            ).then_inc(sE, 16)
            sync.wait_ge(vs, 4)
            sync.dma_start(out=out_d[:], in_=total[:]).then_inc(st, 16)
            sync.wait_ge(st, 32)

        @block.scalar
        def _(scalar):
            scalar.dma_start(
                out=ais[:, :], in_=xis[:].rearrange("(p f) -> p f", f=FREE_A)
            ).then_inc(si, 16)
            scalar.dma_start(
                out=aem[:, :], in_=xem[:].rearrange("(p f) -> p f", f=FREE_A)
            ).then_inc(se, 16)

        @block.vector
        def _(vector):
            # w = m + 1.5*tr_m  (fp8 operands, bf16 out)
            vector.wait_ge(st, 16)
            nc.vector.scalar_tensor_tensor(
                out=w2[:, :], in0=tr_ap, scalar=1.5, in1=m_ap,
                op0=Alu.mult, op1=Alu.add,
            ).then_inc(vs, 1)
            vector.wait_ge(vs, 1)  # w2 committed (engines pipeline)
            # fused dots: rowq[:, 0] = sum ais*[w,w,w]; rowq[:, 1] likewise
            for i, (tile, sem) in enumerate([(ais, si), (aem, se)]):
                vector.wait_ge(sem, 16)
                nc.vector.scalar_tensor_tensor(
                    out=prod[:, FREE_A * i : FREE_A * (i + 1)].rearrange(
                        "p (c f) -> p c f", f=BLK
                    ),
                    in0=tile[:, :].rearrange("p (c f) -> p c f", f=BLK),
                    scalar=1.0, in1=w_b,
                    op0=Alu.mult, op1=Alu.mult,
                    accum_out=rowq[:, i : i + 1],
                ).then_inc(vs, 1)
            # relu the 32 per-row dots (in PSUM), accumulate to one scalar
            vector.wait_ge(pe, 1)
            nc.vector.tensor_scalar(
                out=srelu[:], in0=ps[:], scalar1=0.0, scalar2=None,
                op0=Alu.max, op1=Alu.add, accum_out=total[:],
            ).then_inc(vs, 1)

        @block.tensor
        def _(tensor):
            # ps[1, 32] = rowq[:,0]^T @ E + rowq[:,1]^T @ (-E) — fold each
            # row's 4 partition-quarters; En carries the is-em sign
            tensor.wait_ge(sE, 16)  # E landed
            tensor.wait_ge(vs, 2)
            nc.tensor.matmul(ps[:], rowq[:, 0:1], e_ap, start=True, stop=False)
            tensor.wait_ge(vs, 3)
            nc.tensor.matmul(
                ps[:], rowq[:, 1:2], en_ap, start=False, stop=True
            ).then_inc(pe, 1)

    return nc


def pack_inputs(A_is_t, A_is_t_14, A_is_t_28, A_em_t, A_em_t_14, A_em_t_28, m, tr_m):
    idx = np.arange(B)
    bf = ml_dtypes.bfloat16
    f8 = ml_dtypes.float8_e4m3fn

    def blk(a, dt):  # per-core [128, 256] flattening of a [B, D] operand
        return np.ascontiguousarray(a, dtype=np.float32).astype(dt).reshape(
            N_CORES, 128, BLK
        )

    def dblk(a):  # diagonal gather then per-core flatten, in fp8
        return blk(np.asarray(a)[idx, idx], f8)

    E = np.repeat(np.eye(E_COLS, dtype=np.float32), 4, axis=0)
    Xw = np.empty((N_CORES, 128, 2 * BLK), dtype=f8)
    Xw[:, :, 0:BLK] = blk(m, f8)
    Xw[:, :, BLK : 2 * BLK] = blk(tr_m, f8)
    Xe = np.empty((128, 2 * E_COLS), dtype=bf)
    Xe[:, 0:E_COLS] = E.astype(bf)
    Xe[:, E_COLS : 2 * E_COLS] = (-E).astype(bf)

    Xis = np.empty((N_CORES, 128, FREE_A), dtype=f8)
    Xis[:, :, 0 * BLK : 1 * BLK] = dblk(A_is_t)
    Xis[:, :, 1 * BLK : 2 * BLK] = dblk(A_is_t_14)
    Xis[:, :, 2 * BLK : 3 * BLK] = dblk(A_is_t_28)
    Xem = np.empty((N_CORES, 128, FREE_A), dtype=f8)
    Xem[:, :, 0 * BLK : 1 * BLK] = dblk(A_em_t)
    Xem[:, :, 1 * BLK : 2 * BLK] = dblk(A_em_t_14)
    Xem[:, :, 2 * BLK : 3 * BLK] = dblk(A_em_t_28)

    return [
        {
            "xw": Xw[c].ravel(),
            "xe": Xe.ravel(),
            "xis": Xis[c].ravel(),
            "xem": Xem[c].ravel(),
        }
        for c in range(N_CORES)
    ]


def run(in_maps, **kwargs):
    global _NC_CACHE
    if _NC_CACHE is None:
        _NC_CACHE = build_nc()
    return run_bass_kernel_spmd(
        _NC_CACHE, in_maps, core_ids=list(range(N_CORES)), **kwargs
    )


def kernel(**inputs) -> np.ndarray:
    res = run(pack_inputs(**inputs))
    total = 0.4 * sum(float(r["out"][0, 0]) for r in res.results)
    return np.array([total], dtype=np.float32)


# revision 29
# speedup vs baseline: 1.1212x; 1.1212x over previous
"""Contrastive-loss kernel for Trainium2 (8 NeuronCores, SPMD data-parallel).

Math (from the reference):
    diag_A_is = (A_is_t + A_is_t_14 + A_is_t_28)[i, i, :]        # [B, D]
    diag_A_em = (A_em_t + A_em_t_14 + A_em_t_28)[i, i, :]        # [B, D]
    loss = sum_b relu( sum_d (0.4*m + 0.6*tr_m) * (diag_A_is - diag_A_em) )

Only the diagonals A[i, i, :] of the six [B, B, D] tensors are touched
(1/256th of the data).  Sharding strategy: batch-dim data parallel across
the 8 cores — the host gathers the diagonal rows (pure data movement) and
ships each core its 32 rows of the eight [B, D] operands; all arithmetic
runs on-device.  Per-core partial losses are summed on the host (8
scalars).

Precision plan (tolerance 2e-2): everything ships as fp8 e4m3 except the
exact-in-bf16 E constants (whole-pipeline rel err 4.2e-3, host-measured
and HW-verified; gate margin ~4.8x).  The DVE reads fp8 directly — mixed
fp8 x bf16 operands run at full DVE rate (measured).  All accumulation is
fp32 (DVE accumulator / PSUM).

Device-side layout per core (all HWDGE; the two rings pipeline; the sync
ring q1 starts ~0.35 us earlier after body entry and carries the
latency-critical W load, the scalar ring q10 drains faster and carries
the bulk):
  sync ring q1:     wmt [128, 512] = m | tr_m      (fp8, 512 B rows)
                    et  [128, 64]  = E | En        (bf16, 128 B rows)
                    ... out store (4 B) at the end
  scalar ring q10:  ais [128, 768] = is0|is1|is2   (fp8, 768 B rows)
                    aem [128, 768] = em0|em1|em2   (fp8, 768 B rows)
  each [32, 1024] operand block is flattened row-major to [128, 256]
  (partition p = 4*row + quarter, 256 contiguous d's per partition).
  E[p, b] = 1.0 iff p // 4 == b; En = -E.  Both are host CONSTANTS (not
  input data): matmul rhs that folds the four per-partition quarter-row
  dots of each batch row (partition reduction), with En carrying the
  is-minus-em sign so no negated weight vector is ever needed.

Factoring: 0.4*m + 0.6*tr_m = 0.4 * (m + 1.5*tr_m) and
relu(0.4 x) = 0.4 relu(x), so the 0.4 is applied host-side to the scalar.
Linearity: the is- and em-blocks are separately dotted against w
(broadcast across the three tensors via a stride-0 AP — same DVE rate as
linear, measured) in one accumulator pass each:
    rowq[:, 0] = sum_f ais * [w,w,w],   rowq[:, 1] = sum_f aem * [w,w,w]
    ps[1, 32]  = rowq[:, 0]^T @ E  +  rowq[:, 1]^T @ En
This removes the -w preparation from the serial DVE chain entirely: the
chain is w-prep, dot(is), dot(em), relu — everything else overlaps.

relu+sum runs on DVE (the Scalar engine's ACT path lazily loads a 1.3 us
function table on first use — measured, avoid).  The 4-byte result store
is issued from the sync ring (cheaper enqueue there; enqueue+flight is
~1.6 us on either ring).

Raw bass (no TileContext) on purpose: this walrus build enforces a tiny
per-instruction sync-wait limit and Tile's epilogue barrier costs several
microseconds.  Engines pipeline, so a same-engine consumer of an earlier
op's output still needs a semaphore edge (the race detector enforces it).
The Pool engine cannot run TensorScalarPtr (codegen engine check) and
SWDGE completion semaphores lag ~1 us behind the last byte, so everything
stays on the two HWDGE rings.
"""

import numpy as np
import ml_dtypes

import concourse.bass as bass
import concourse.mybir as mybir
from concourse.bass_utils import run_bass_kernel_spmd

B = 256
D = 1024
N_CORES = 8
ROWS_PER_CORE = B // N_CORES  # 32
BLK = 256  # free-dim width of one packed [32, 1024] operand block
E_COLS = ROWS_PER_CORE  # 32
FREE_A = 3 * BLK  # 768: is or em block

_NC_CACHE = None


def build_nc() -> bass.Bass:
    f32 = mybir.dt.float32
    bf16 = mybir.dt.bfloat16
    f8 = mybir.dt.float8e4
    Alu = mybir.AluOpType

    nc = bass.Bass()
    xw = nc.dram_tensor("xw", [128 * 2 * BLK], f8, kind="ExternalInput")
    xe = nc.dram_tensor("xe", [128 * 2 * E_COLS], bf16, kind="ExternalInput")
    xis = nc.dram_tensor("xis", [128 * FREE_A], f8, kind="ExternalInput")
    xem = nc.dram_tensor("xem", [128 * FREE_A], f8, kind="ExternalInput")
    out_d = nc.dram_tensor("out", [1, 1], f32, kind="ExternalOutput")

    with (
        nc.sbuf_tensor("wmt", [128, 2 * BLK], f8) as wmt,
        nc.sbuf_tensor("et", [128, 2 * E_COLS], bf16) as et,
        nc.sbuf_tensor("ais", [128, FREE_A], f8) as ais,
        nc.sbuf_tensor("aem", [128, FREE_A], f8) as aem,
        nc.sbuf_tensor("w2", [128, BLK], bf16) as w2,
        nc.sbuf_tensor("prod", [128, 2 * FREE_A], bf16) as prod,
        nc.sbuf_tensor("rowq", [128, 2], bf16) as rowq,
        nc.sbuf_tensor("srelu", [1, E_COLS], f32) as srelu,
        nc.sbuf_tensor("total", [1, 1], f32) as total,
        nc.psum_tensor("ps", [1, E_COLS], f32) as ps,
        nc.semaphore("st") as st,  # scalar ring: m|tr (16), out (32)
        nc.semaphore("sE") as sE,  # scalar ring: E|En load
        nc.semaphore("se") as se,  # sync ring: em block
        nc.semaphore("si") as si,  # sync ring: is block
        nc.semaphore("vs") as vs,  # vector progress
        nc.semaphore("pe") as pe,  # tensor: partition fold done
        nc.Block() as block,
    ):
        m_ap = wmt[:, 0:BLK]
        tr_ap = wmt[:, BLK : 2 * BLK]
        e_ap = et[:, 0:E_COLS]
        en_ap = et[:, E_COLS : 2 * E_COLS]
        w_b = w2[:, :].unsqueeze(1).broadcast_to([128, 3, BLK])

        @block.sync
        def _(sync):
            sync.dma_start(
                out=wmt[:, :], in_=xw[:].rearrange("(p f) -> p f", f=2 * BLK)
            ).then_inc(st, 16)
            sync.dma_start(
                out=et[:, :], in_=xe[:].rearrange("(p f) -> p f", f=2 * E_COLS)
            ).then_inc(sE, 16)
            sync.dma_start(
                out=aem[:, :], in_=xem[:].rearrange("(p f) -> p f", f=FREE_A)
            ).then_inc(se, 16)

        @block.scalar
        def _(scalar):
            scalar.dma_start(
                out=ais[:, :], in_=xis[:].rearrange("(p f) -> p f", f=FREE_A)
            ).then_inc(si, 16)
            # out store as the scalar ring's 2nd enqueue: ACT's 2nd
            # DMA_DIRECT2D issues in ~825 ns and q10's 4 B flight is ~500 ns
            # (vs 666+930 on the sync ring) — measured
            scalar.wait_ge(vs, 4)
            scalar.dma_start(out=out_d[:], in_=total[:]).then_inc(st, 16)
            scalar.wait_ge(st, 32)

        @block.vector
        def _(vector):
            # w = m + 1.5*tr_m  (fp8 operands, bf16 out)
            vector.wait_ge(st, 16)
            nc.vector.scalar_tensor_tensor(
                out=w2[:, :], in0=tr_ap, scalar=1.5, in1=m_ap,
                op0=Alu.mult, op1=Alu.add,
            ).then_inc(vs, 1)
            vector.wait_ge(vs, 1)  # w2 committed (engines pipeline)
            # fused dots: rowq[:, 0] = sum ais*[w,w,w]; rowq[:, 1] likewise
            for i, (tile, sem) in enumerate([(ais, si), (aem, se)]):
                vector.wait_ge(sem, 16)
                nc.vector.scalar_tensor_tensor(
                    out=prod[:, FREE_A * i : FREE_A * (i + 1)].rearrange(
                        "p (c f) -> p c f", f=BLK
                    ),
                    in0=tile[:, :].rearrange("p (c f) -> p c f", f=BLK),
                    scalar=1.0, in1=w_b,
                    op0=Alu.mult, op1=Alu.mult,
                    accum_out=rowq[:, i : i + 1],
                ).then_inc(vs, 1)
            # relu the 32 per-row dots (in PSUM), accumulate to one scalar
            vector.wait_ge(pe, 1)
            nc.vector.tensor_scalar(
                out=srelu[:], in0=ps[:], scalar1=0.0, scalar2=None,
                op0=Alu.max, op1=Alu.add, accum_out=total[:],
            ).then_inc(vs, 1)

        @block.tensor
        def _(tensor):
            # ps[1, 32] = rowq[:,0]^T @ E + rowq[:,1]^T @ (-E) — fold each
            # row's 4 partition-quarters; En carries the is-em sign
            tensor.wait_ge(sE, 16)  # E landed
            tensor.wait_ge(vs, 2)
            nc.tensor.matmul(ps[:], rowq[:, 0:1], e_ap, start=True, stop=False)
            tensor.wait_ge(vs, 3)
            nc.tensor.matmul(
                ps[:], rowq[:, 1:2], en_ap, start=False, stop=True
            ).then_inc(pe, 1)

    return nc


def pack_inputs(A_is_t, A_is_t_14, A_is_t_28, A_em_t, A_em_t_14, A_em_t_28, m, tr_m):
    idx = np.arange(B)
    bf = ml_dtypes.bfloat16
    f8 = ml_dtypes.float8_e4m3fn

    def blk(a, dt):  # per-core [128, 256] flattening of a [B, D] operand
        return np.ascontiguousarray(a, dtype=np.float32).astype(dt).reshape(
            N_CORES, 128, BLK
        )

    def dblk(a):  # diagonal gather then per-core flatten, in fp8
        return blk(np.asarray(a)[idx, idx], f8)

    E = np.repeat(np.eye(E_COLS, dtype=np.float32), 4, axis=0)
    Xw = np.empty((N_CORES, 128, 2 * BLK), dtype=f8)
    Xw[:, :, 0:BLK] = blk(m, f8)
    Xw[:, :, BLK : 2 * BLK] = blk(tr_m, f8)
    Xe = np.empty((128, 2 * E_COLS), dtype=bf)
    Xe[:, 0:E_COLS] = E.astype(bf)
    Xe[:, E_COLS : 2 * E_COLS] = (-E).astype(bf)

    Xis = np.empty((N_CORES, 128, FREE_A), dtype=f8)
    Xis[:, :, 0 * BLK : 1 * BLK] = dblk(A_is_t)
    Xis[:, :, 1 * BLK : 2 * BLK] = dblk(A_is_t_14)
    Xis[:, :, 2 * BLK : 3 * BLK] = dblk(A_is_t_28)
    Xem = np.empty((N_CORES, 128, FREE_A), dtype=f8)
    Xem[:, :, 0 * BLK : 1 * BLK] = dblk(A_em_t)
    Xem[:, :, 1 * BLK : 2 * BLK] = dblk(A_em_t_14)
    Xem[:, :, 2 * BLK : 3 * BLK] = dblk(A_em_t_28)

    return [
        {
            "xw": Xw[c].ravel(),
            "xe": Xe.ravel(),
            "xis": Xis[c].ravel(),
            "xem": Xem[c].ravel(),
        }
        for c in range(N_CORES)
    ]


def run(in_maps, **kwargs):
    global _NC_CACHE
    if _NC_CACHE is None:
        _NC_CACHE = build_nc()
    return run_bass_kernel_spmd(
        _NC_CACHE, in_maps, core_ids=list(range(N_CORES)), **kwargs
    )


def kernel(**inputs) -> np.ndarray:
    res = run(pack_inputs(**inputs))
    total = 0.4 * sum(float(r["out"][0, 0]) for r in res.results)
    return np.array([total], dtype=np.float32)


# revision 30
# speedup vs baseline: 1.1293x; 1.0072x over previous
"""Contrastive-loss kernel for Trainium2 (8 NeuronCores, SPMD data-parallel).

Math (from the reference):
    diag_A_is = (A_is_t + A_is_t_14 + A_is_t_28)[i, i, :]        # [B, D]
    diag_A_em = (A_em_t + A_em_t_14 + A_em_t_28)[i, i, :]        # [B, D]
    loss = sum_b relu( sum_d (0.4*m + 0.6*tr_m) * (diag_A_is - diag_A_em) )

Only the diagonals A[i, i, :] of the six [B, B, D] tensors are touched
(1/256th of the data).  Sharding strategy: batch-dim data parallel across
the 8 cores — the host gathers the diagonal rows (pure data movement) and
ships each core its 32 rows of the eight [B, D] operands; all arithmetic
runs on-device.  Per-core partial losses are summed on the host (8
scalars).

Precision plan (tolerance 2e-2): everything ships as fp8 e4m3 except the
exact-in-bf16 E constants (whole-pipeline rel err 4.2e-3, host-measured
and HW-verified; gate margin ~4.8x).  The DVE reads fp8 directly — mixed
fp8 x bf16 operands run at full DVE rate (measured).  All accumulation is
fp32 (DVE accumulator / PSUM).

Device-side layout per core (all HWDGE; the two rings pipeline; the sync
ring q1 starts ~0.35 us earlier after body entry and carries the
latency-critical W load, the scalar ring q10 drains faster and carries
the bulk):
  sync ring q1:     wmt [128, 512] = m | tr_m      (fp8, 512 B rows)
                    et  [128, 64]  = E | En        (bf16, 128 B rows)
                    ... out store (4 B) at the end
  scalar ring q10:  ais [128, 768] = is0|is1|is2   (fp8, 768 B rows)
                    aem [128, 768] = em0|em1|em2   (fp8, 768 B rows)
  each [32, 1024] operand block is flattened row-major to [128, 256]
  (partition p = 4*row + quarter, 256 contiguous d's per partition).
  E[p, b] = 1.0 iff p // 4 == b; En = -E.  Both are host CONSTANTS (not
  input data): matmul rhs that folds the four per-partition quarter-row
  dots of each batch row (partition reduction), with En carrying the
  is-minus-em sign so no negated weight vector is ever needed.

Factoring: 0.4*m + 0.6*tr_m = 0.4 * (m + 1.5*tr_m) and
relu(0.4 x) = 0.4 relu(x), so the 0.4 is applied host-side to the scalar.
Linearity: the is- and em-blocks are separately dotted against w
(broadcast across the three tensors via a stride-0 AP — same DVE rate as
linear, measured) in one accumulator pass each:
    rowq[:, 0] = sum_f ais * [w,w,w],   rowq[:, 1] = sum_f aem * [w,w,w]
    ps[1, 32]  = rowq[:, 0]^T @ E  +  rowq[:, 1]^T @ En
This removes the -w preparation from the serial DVE chain entirely: the
chain is w-prep, dot(is), dot(em), relu — everything else overlaps.

relu+sum runs on DVE (the Scalar engine's ACT path lazily loads a 1.3 us
function table on first use — measured, avoid).  The 4-byte result store
is issued from the sync ring (cheaper enqueue there; enqueue+flight is
~1.6 us on either ring).

Raw bass (no TileContext) on purpose: this walrus build enforces a tiny
per-instruction sync-wait limit and Tile's epilogue barrier costs several
microseconds.  Engines pipeline, so a same-engine consumer of an earlier
op's output still needs a semaphore edge (the race detector enforces it).
The Pool engine cannot run TensorScalarPtr (codegen engine check) and
SWDGE completion semaphores lag ~1 us behind the last byte, so everything
stays on the two HWDGE rings.
"""

import numpy as np
import ml_dtypes

import concourse.bass as bass
import concourse.mybir as mybir
from concourse.bass_utils import run_bass_kernel_spmd

B = 256
D = 1024
N_CORES = 8
ROWS_PER_CORE = B // N_CORES  # 32
BLK = 256  # free-dim width of one packed [32, 1024] operand block
E_COLS = ROWS_PER_CORE  # 32
FREE_A = 3 * BLK  # 768: is or em block

_NC_CACHE = None


def build_nc() -> bass.Bass:
    f32 = mybir.dt.float32
    bf16 = mybir.dt.bfloat16
    f8 = mybir.dt.float8e4
    Alu = mybir.AluOpType

    nc = bass.Bass()
    xw = nc.dram_tensor("xw", [128 * 2 * BLK], f8, kind="ExternalInput")
    xe = nc.dram_tensor("xe", [128 * 2 * E_COLS], bf16, kind="ExternalInput")
    xis = nc.dram_tensor("xis", [128 * FREE_A], f8, kind="ExternalInput")
    xem = nc.dram_tensor("xem", [128 * FREE_A], f8, kind="ExternalInput")
    out_d = nc.dram_tensor("out", [1, 1], f32, kind="ExternalOutput")

    with (
        nc.sbuf_tensor("wmt", [128, 2 * BLK], f8) as wmt,
        nc.sbuf_tensor("et", [128, 2 * E_COLS], bf16) as et,
        nc.sbuf_tensor("ais", [128, FREE_A], f8) as ais,
        nc.sbuf_tensor("aem", [128, FREE_A], f8) as aem,
        nc.sbuf_tensor("w2", [128, BLK], bf16) as w2,
        nc.sbuf_tensor("prod", [128, 2 * FREE_A], bf16) as prod,
        nc.sbuf_tensor("rowq", [128, 2], bf16) as rowq,
        nc.sbuf_tensor("srelu", [1, E_COLS], f32) as srelu,
        nc.sbuf_tensor("total", [1, 1], f32) as total,
        nc.psum_tensor("ps", [1, E_COLS], f32) as ps,
        nc.semaphore("st") as st,  # scalar ring: m|tr (16), out (32)
        nc.semaphore("sE") as sE,  # scalar ring: E|En load
        nc.semaphore("se") as se,  # sync ring: em block
        nc.semaphore("si") as si,  # sync ring: is block
        nc.semaphore("vs") as vs,  # vector progress
        nc.semaphore("pe") as pe,  # tensor: partition fold done
        nc.Block() as block,
    ):
        m_ap = wmt[:, 0:BLK]
        tr_ap = wmt[:, BLK : 2 * BLK]
        e_ap = et[:, 0:E_COLS]
        en_ap = et[:, E_COLS : 2 * E_COLS]
        w_b = w2[:, :].unsqueeze(1).broadcast_to([128, 3, BLK])

        @block.sync
        def _(sync):
            sync.dma_start(
                out=wmt[:, :], in_=xw[:].rearrange("(p f) -> p f", f=2 * BLK)
            ).then_inc(st, 16)
            sync.dma_start(
                out=et[:, :], in_=xe[:].rearrange("(p f) -> p f", f=2 * E_COLS)
            ).then_inc(sE, 16)
            sync.wait_ge(vs, 4)
            sync.dma_start(out=out_d[:], in_=total[:]).then_inc(st, 16)
            sync.wait_ge(st, 32)

        @block.scalar
        def _(scalar):
            scalar.dma_start(
                out=ais[:, :], in_=xis[:].rearrange("(p f) -> p f", f=FREE_A)
            ).then_inc(si, 16)
            scalar.dma_start(
                out=aem[:, :], in_=xem[:].rearrange("(p f) -> p f", f=FREE_A)
            ).then_inc(se, 16)

        @block.vector
        def _(vector):
            # w = m + 1.5*tr_m  (fp8 operands, bf16 out)
            vector.wait_ge(st, 16)
            nc.vector.scalar_tensor_tensor(
                out=w2[:, :], in0=tr_ap, scalar=1.5, in1=m_ap,
                op0=Alu.mult, op1=Alu.add,
            ).then_inc(vs, 1)
            vector.wait_ge(vs, 1)  # w2 committed (engines pipeline)
            # fused dots: rowq[:, 0] = sum ais*[w,w,w]; rowq[:, 1] likewise
            for i, (tile, sem) in enumerate([(ais, si), (aem, se)]):
                vector.wait_ge(sem, 16)
                nc.vector.scalar_tensor_tensor(
                    out=prod[:, FREE_A * i : FREE_A * (i + 1)].rearrange(
                        "p (c f) -> p c f", f=BLK
                    ),
                    in0=tile[:, :].rearrange("p (c f) -> p c f", f=BLK),
                    scalar=1.0, in1=w_b,
                    op0=Alu.mult, op1=Alu.mult,
                    accum_out=rowq[:, i : i + 1],
                ).then_inc(vs, 1)
            # relu the 32 per-row dots (in PSUM), accumulate to one scalar
            vector.wait_ge(pe, 1)
            nc.vector.tensor_scalar(
                out=srelu[:], in0=ps[:], scalar1=0.0, scalar2=None,
                op0=Alu.max, op1=Alu.add, accum_out=total[:],
            ).then_inc(vs, 1)

        @block.tensor
        def _(tensor):
            # ps[1, 32] = rowq[:,0]^T @ E + rowq[:,1]^T @ (-E) — fold each
            # row's 4 partition-quarters; En carries the is-em sign
            tensor.wait_ge(sE, 16)  # E landed
            tensor.wait_ge(vs, 2)
            nc.tensor.matmul(ps[:], rowq[:, 0:1], e_ap, start=True, stop=False)
            tensor.wait_ge(vs, 3)
            nc.tensor.matmul(
                ps[:], rowq[:, 1:2], en_ap, start=False, stop=True
            ).then_inc(pe, 1)

    return nc


def pack_inputs(A_is_t, A_is_t_14, A_is_t_28, A_em_t, A_em_t_14, A_em_t_28, m, tr_m):
    idx = np.arange(B)
    bf = ml_dtypes.bfloat16
    f8 = ml_dtypes.float8_e4m3fn

    def blk(a, dt):  # per-core [128, 256] flattening of a [B, D] operand
        return np.ascontiguousarray(a, dtype=np.float32).astype(dt).reshape(
            N_CORES, 128, BLK
        )

    def dblk(a):  # diagonal gather then per-core flatten, in fp8
        return blk(np.asarray(a)[idx, idx], f8)

    E = np.repeat(np.eye(E_COLS, dtype=np.float32), 4, axis=0)
    Xw = np.empty((N_CORES, 128, 2 * BLK), dtype=f8)
    Xw[:, :, 0:BLK] = blk(m, f8)
    Xw[:, :, BLK : 2 * BLK] = blk(tr_m, f8)
    Xe = np.empty((128, 2 * E_COLS), dtype=bf)
    Xe[:, 0:E_COLS] = E.astype(bf)
    Xe[:, E_COLS : 2 * E_COLS] = (-E).astype(bf)

    Xis = np.empty((N_CORES, 128, FREE_A), dtype=f8)
    Xis[:, :, 0 * BLK : 1 * BLK] = dblk(A_is_t)
    Xis[:, :, 1 * BLK : 2 * BLK] = dblk(A_is_t_14)
    Xis[:, :, 2 * BLK : 3 * BLK] = dblk(A_is_t_28)
    Xem = np.empty((N_CORES, 128, FREE_A), dtype=f8)
    Xem[:, :, 0 * BLK : 1 * BLK] = dblk(A_em_t)
    Xem[:, :, 1 * BLK : 2 * BLK] = dblk(A_em_t_14)
    Xem[:, :, 2 * BLK : 3 * BLK] = dblk(A_em_t_28)

    return [
        {
            "xw": Xw[c].ravel(),
            "xe": Xe.ravel(),
            "xis": Xis[c].ravel(),
            "xem": Xem[c].ravel(),
        }
        for c in range(N_CORES)
    ]


def run(in_maps, **kwargs):
    global _NC_CACHE
    if _NC_CACHE is None:
        _NC_CACHE = build_nc()
    return run_bass_kernel_spmd(
        _NC_CACHE, in_maps, core_ids=list(range(N_CORES)), **kwargs
    )


def kernel(**inputs) -> np.ndarray:
    res = run(pack_inputs(**inputs))
    total = 0.4 * sum(float(r["out"][0, 0]) for r in res.results)
    return np.array([total], dtype=np.float32)
